# revision 25
# baseline (speedup 1.0000x reference)
"""Trainium2 Bass kernel for nn_Attention (B=4, C=512, T=8, H=14, W=14).

Math (see reference): tokens tok[b, n, c] with n = t*H*W + h*W + w, c channel.
q = k = v = tok split into 8 heads of d=64 where head hd takes channels
c = d*8 + hd (strided!).  Per (b, hd):  S = q q^T / 64,  P = softmax_rows(S),
out = P q.  Output back to [B, C, T, H, W].

Key identities used:
 - x viewed as [B, C, N] gives qT (the [d, N] layout of a head) as the row
   slice x[b, hd::8, :].  Output wants the same [d, N] layout.
 - S is symmetric (q == k), so E = exp(S/64) is symmetric; block-ROWS of E
   (as produced) serve as the [m_contraction, n_free] operand of the second
   matmul without any transposes:
       outT[d, n] = sum_m q_nd[m, d] * E[m, n]   (= (P q)^T * rowsum)
 - softmax normalization: outT[:, n] *= 1/rowsum[n] -- a free-axis broadcast,
   materialized via a DRAM round trip with a partition-stride-0 gather DMA.
 - exp needs no max-subtraction: S/64 is in ~[-1.5, 2.5] for randn inputs.

Sharding: 32 independent (b, hd) units; core c gets b = c//2,
heads 4*(c%2) .. +4.

ACT (ScalarE) exp is the bottleneck engine, so the PSUM layout is chosen to
minimize exp instruction count (each ACTIVATE pays ~170-350 cycles of fixed
overhead).  PSUM budget (8 banks, the binding constraint):
 - tag "s":  [128, 1536] f32 S-tiles (3 banks) x bufs=2 = 6 banks -- ONE exp
   per 128-row block covers 1536 of its 1568 columns, accum_out produces the
   softmax denominator for free.
 - tag "st": [128, 13*32] f32 (1 bank): the 32 trailing S columns of all 13
   blocks, ONE exp + one DVE reduce for the lot.  Its matmuls are bunched at
   the END of phase A so head h+1's PE stream never waits on head h's tail
   exp (interleaving them per-block serializes PE behind ACT -- measured
   ~2x slower).
 - tag "o":  1 bank shared by the packed q_nd transpose target and the
   out'-accumulator quarters [64, 512].  Both live in phase B only; placing
   the transposes in phase A stalls PE at head boundaries.

Engines: PE: S-matmuls (K=64, bf16) + outT-matmuls (K=128, bf16) + layout
transposes; ACT: 14 exps/head; DVE: tail reduce, reciprocal, 32x32
transpose, final normalize; DMA: contiguous loads/stores + the normalizer
round trip (strided 4-byte-run DMAs are catastrophically slow -- every DMA
here moves >=128B contiguous runs).  Heads are software-pipelined (phase A
of head h+1 emitted before phase B of head h) so ACT never drains.

Measured (in-NEFF repeat-loop wall-clock slope, drift-controlled A/B):
~64 us per full problem across 8 cores; ACT-roofline-bound.
"""

import sys

if "/opt/trn_rl_repo" not in sys.path:
    sys.path.insert(0, "/opt/trn_rl_repo")

import ml_dtypes
import numpy as np

import concourse.bass as bass
import concourse.mybir as mybir
import concourse.tile as tile
from concourse import bacc, bass_utils

B, CH, T, H, W = 4, 512, 8, 14, 14
N = T * H * W            # 1568 tokens
D = 64                   # head size
NHEADS = 8
N_CORES = 8
HPC = 4                  # heads per core
BLK = 128
NBLK = (N + BLK - 1) // BLK   # 13
M_LAST = N - (NBLK - 1) * BLK  # 32
NMAIN = 1536             # 3 psum banks of S columns per block
NTAIL = N - NMAIN        # 32 trailing columns, collected across blocks

_BF16 = mybir.dt.bfloat16
_F32 = mybir.dt.float32

LAST_RESULT = None  # BassKernelResults of the most recent run (for test.py)
_NC_CACHE = None


def _build_nc(nrep: int = 1):
    from contextlib import ExitStack

    from concourse.masks import make_identity

    # Bacc (not plain Bass): its compile() runs move_matmul_waits_to_ldweights
    # + generate_event_semaphores, required to satisfy the 1-wait-per-
    # instruction hardware constraint that walrus enforces.
    nc = bacc.Bacc("TRN2")
    q_dram = nc.dram_tensor("q", [HPC, D, N], _BF16, kind="ExternalInput").ap()
    o_dram = nc.dram_tensor("o", [HPC, D, N], _F32, kind="ExternalOutput").ap()

    with tile.TileContext(nc) as tc:
        with (
            tc.tile_pool(name="ps", bufs=2, space="PSUM") as ps,
            tc.tile_pool(name="sb", bufs=2) as sb,
            tc.tile_pool(name="epool", bufs=39) as epool,
            tc.tile_pool(name="small", bufs=3) as small,
            tc.tile_pool(name="singles", bufs=1) as singles,
            tc.tile_pool(name="scr", bufs=2, space="DRAM") as scr,
            ExitStack() as rep_ctx,
        ):
            ident = singles.tile([D, D], _BF16, name="ident")
            make_identity(nc, ident)

            if nrep > 1:  # timing mode: repeat the whole program in-NEFF
                rep_ctx.enter_context(tc.For_i(0, nrep, 1))

            def phase_a(h):
                """Load, S matmuls, exp+rowsums, 1/rowsum broadcast."""
                qT = sb.tile([D, N], _BF16, tag="qT", bufs=3, name=f"qT_{h}")
                nc.sync.dma_start(out=qT, in_=q_dram[h])

                rowsums = small.tile([BLK, NBLK], _F32, tag="rs", name=f"rs_{h}")
                nc.vector.memset(rowsums, 1.0)

                e_tiles = []
                for k in range(NBLK):
                    mk = BLK if k < NBLK - 1 else M_LAST
                    ek = epool.tile([BLK, NMAIN], _BF16, tag="e",
                                    name=f"e_{h}_{k}")
                    lhsT = qT[:, k * BLK : k * BLK + mk]
                    s_ps = ps.tile([BLK, NMAIN], _F32, tag="s",
                                   name=f"s_{h}_{k}")
                    for c in range(3):
                        nc.tensor.matmul(
                            s_ps[0:mk, c * 512 : (c + 1) * 512], lhsT,
                            qT[:, c * 512 : (c + 1) * 512],
                            start=True, stop=True,
                        )
                    nc.scalar.activation(
                        ek[0:mk, :], s_ps[0:mk, :],
                        mybir.ActivationFunctionType.Exp,
                        scale=1.0 / 64.0,
                        accum_out=rowsums[0:mk, k : k + 1],
                    )
                    e_tiles.append((ek, mk))

                # 32-col tails of all 13 blocks, bunched at the END of the
                # phase so head h+1's PE stream never waits on this exp.
                st_ps = ps.tile([BLK, NBLK * NTAIL], _F32, tag="st", bufs=1,
                                name=f"st_{h}")
                nc.vector.memset(st_ps, 0.0)  # rows 32:128 of k=12: no inf
                for k in range(NBLK):
                    mk = BLK if k < NBLK - 1 else M_LAST
                    nc.tensor.matmul(
                        st_ps[0:mk, k * NTAIL : (k + 1) * NTAIL],
                        qT[:, k * BLK : k * BLK + mk], qT[:, NMAIN:N],
                        start=True, stop=True,
                    )
                etail = sb.tile([BLK, NBLK * NTAIL], _BF16, tag="et",
                                name=f"et_{h}")
                nc.scalar.activation(
                    etail, st_ps, mybir.ActivationFunctionType.Exp,
                    scale=1.0 / 64.0,
                )
                tails = small.tile([BLK, NBLK], _F32, tag="ts", name=f"ts_{h}")
                nc.vector.tensor_reduce(
                    out=tails,
                    in_=etail.rearrange("p (k t) -> p k t", t=NTAIL),
                    axis=mybir.AxisListType.X,
                    op=mybir.AluOpType.add,
                )
                nc.vector.tensor_add(rowsums, rowsums, tails)

                # normalizer row 1/rowsum broadcast to [D, N]: DVE 32x32
                # block-transpose puts recip[p, j] at rt[32a+j, p%32], so each
                # 32-partition band writes scratch contiguously; the read-back
                # replicates the row via a stride-0 partition dim.
                recip = small.tile([BLK, 32], _F32, tag="recip", name=f"rc_{h}")
                nc.vector.reciprocal(recip[:, 0:NBLK], rowsums)
                rt = small.tile([BLK, 32], _F32, tag="rt", name=f"rt_{h}")
                nc.vector.transpose(rt, recip)
                scratch = scr.tile([NBLK * BLK], _F32, tag="v", name=f"scr_{h}")
                for a in range(4):
                    # scratch[j*128 + 32a + i] = rt[32a+j, i] = recip[32a+i, j]
                    nc.sync.dma_start(
                        out=bass.AP(
                            tensor=scratch.tensor,
                            offset=scratch.offset + 32 * a,
                            ap=[[BLK, NBLK], [1, 32]],
                        ),
                        in_=rt[32 * a : 32 * a + NBLK, :],
                    )
                rsb = sb.tile([D, N], _F32, tag="R", bufs=3, name=f"R_{h}")
                nc.gpsimd.dma_start(
                    out=rsb,
                    in_=bass.AP(
                        tensor=scratch.tensor,
                        offset=scratch.offset,
                        ap=[[0, D], [1, N]],
                    ),
                )
                return qT, e_tiles, etail, rsb

            def phase_b(h, state):
                """q_nd transposes, outT = sum_k qn_k^T @ E_k, normalize,
                store.  The transposes live here (not phase A) so the shared
                "o" psum slot is only contended within this phase."""
                qT, e_tiles, etail, rsb = state
                qn = sb.tile([BLK, NBLK * D], _BF16, tag="qn", bufs=2,
                             name=f"qn_{h}")
                qnp = ps.tile([BLK, NBLK * D], _BF16, tag="o", bufs=1,
                              name=f"qnp_{h}")
                for k in range(NBLK):
                    mk = BLK if k < NBLK - 1 else M_LAST
                    nc.tensor.transpose(
                        qnp[0:mk, k * D : (k + 1) * D],
                        qT[:, k * BLK : k * BLK + mk], ident,
                    )
                nc.vector.tensor_copy(qn, qnp)

                outT = sb.tile([D, N], _F32, tag="outT", name=f"outT_{h}")
                for quarter in range(4):
                    base = quarter * 512
                    width = 512 if quarter < 3 else NTAIL
                    op = ps.tile([D, width], _F32, tag="o", bufs=1,
                                 name=f"o_{h}_{quarter}")
                    for k in range(NBLK):
                        mk = BLK if k < NBLK - 1 else M_LAST
                        ek, _ = e_tiles[k]
                        lhsT = qn[0:mk, k * D : (k + 1) * D]
                        rhs = (
                            ek[0:mk, base : base + width]
                            if quarter < 3
                            else etail[0:mk, k * NTAIL : (k + 1) * NTAIL]
                        )
                        nc.tensor.matmul(
                            op, lhsT, rhs,
                            start=(k == 0), stop=(k == NBLK - 1),
                        )
                    nc.vector.tensor_mul(
                        outT[:, base : base + width], op,
                        rsb[:, base : base + width],
                    )
                nc.sync.dma_start(out=o_dram[h], in_=outT)

            # software pipeline: A(0) A(1) B(0) A(2) B(1) A(3) B(2) B(3) —
            # PE's S-matmuls for head h+1 are queued before B(h), so ACT's
            # exp stream never drains.
            states = {}
            states[0] = phase_a(0)
            for h in range(1, HPC):
                states[h] = phase_a(h)
                phase_b(h - 1, states.pop(h - 1))
            phase_b(HPC - 1, states.pop(HPC - 1))

    nc.compile()
    return nc


def _prep_inputs(x: np.ndarray) -> list:
    # channel c = d*8 + hd  ->  view [B, D, NHEADS, N]
    xr = np.asarray(x).reshape(B, D, NHEADS, N)
    in_maps = []
    for c in range(N_CORES):
        b, h0 = c // 2, HPC * (c % 2)
        q_t = np.ascontiguousarray(
            xr[b, :, h0 : h0 + HPC, :].transpose(1, 0, 2)
        )  # [HPC, D, N] fp32
        in_maps.append({"q": q_t.astype(ml_dtypes.bfloat16)})
    return in_maps


def kernel(x: np.ndarray) -> np.ndarray:
    global LAST_RESULT, _NC_CACHE
    assert x.shape == (B, CH, T, H, W) and x.dtype == np.float32
    if _NC_CACHE is None:
        _NC_CACHE = _build_nc()
    nc = _NC_CACHE

    in_maps = _prep_inputs(x)
    # The devices intermittently report NRT_EXEC_UNIT_UNRECOVERABLE on a
    # first execute (wedged state from a prior process); a retry clears it.
    last_exc = None
    for attempt in range(3):
        try:
            LAST_RESULT = bass_utils.run_bass_kernel_spmd(
                nc, in_maps, core_ids=list(range(N_CORES))
            )
            break
        except Exception as e:  # noqa: BLE001
            last_exc = e
            import time as _time

            _time.sleep(2.0 + 3.0 * attempt)
    else:
        raise last_exc

    full = np.empty((B, D, NHEADS, N), np.float32)
    for c in range(N_CORES):
        b, h0 = c // 2, HPC * (c % 2)
        o = LAST_RESULT.results[c]["o"]  # [HPC, D, N]
        full[b, :, h0 : h0 + HPC, :] = o.transpose(1, 0, 2)
    return full.reshape(B, CH, T, H, W)
